# revision 3
# baseline (speedup 1.0000x reference)
"""Trainium2 Bass kernel for the interaction-network GNN (nn_Physics_7370163880185).

Reference computation (per batch element b, objects i=0..15, D=256):
  trans  = MLP_t(objs)                              # 256->512->512->256, relu x3
  pair(i,j) = concat(objs_i, objs_j)                # [512]
  inter  = MLP_i(pair)                              # 512->512->512->256, relu x3
  out    = trans + sum_{j != i} inter(i,j) + objs

Sharding: data-parallel over batch B=512 across 8 cores (64 per core).

Kernel strategy (per core):
  * Everything is computed in a feature-on-partition layout: activations are
    [feat, rows] with rows = (n, b) flattened; matmul(out, lhsT=W[k,m],
    rhs=xT[k, rows]) chains layers without transposes.
  * Interaction layer 1 is split: concat(a,b) @ iW1 = a @ iW1[:256] + b @ iW1[256:],
    so U = objs @ iW1[:256] and V = objs @ iW1[256:] are computed once on
    N*B rows instead of N^2*B rows; h1(i,j) = relu(U_i + V_j + ib1) is a
    broadcast add on the vector engine.
  * Matmuls run in float32r (full-rate fp32 on TRN2 PE for free dim >= 256).
  * The j-sum uses a log2 tree reduction with the diagonal block zeroed.
"""

import numpy as np

import concourse.bass as bass
import concourse.mybir as mybir
import concourse.tile as tile
from concourse import bacc
from concourse.bass_utils import run_bass_kernel_spmd

N = 16
B = 512
D = 256
NCORES = 8
BL = B // NCORES          # 64 batch rows per core
ROWS = N * BL             # 1024 (n, b) rows per core
PT = 128                  # partition tile
NT = 512                  # moving-dim (free) tile for matmuls

F32 = mybir.dt.float32
F32R = mybir.dt.float32r
RELU = mybir.ActivationFunctionType.Relu
IDENT = mybir.ActivationFunctionType.Identity
ADD = mybir.AluOpType.add
MAX = mybir.AluOpType.max

WEIGHT_SHAPES = {
    "tW1": (256, 512), "tW2": (512, 512), "tW3": (512, 256),
    "iW1": (512, 512), "iW2": (512, 512), "iW3": (512, 256),
}
BIAS_SHAPES = {"tb1": 512, "tb2": 512, "tb3": 256,
               "ib1": 512, "ib2": 512, "ib3": 256}


def _build_body(nc, tc, prm, ctx):
    cpool = ctx.enter_context(tc.tile_pool(name="const", bufs=1))
    wpool = ctx.enter_context(tc.tile_pool(name="work", bufs=2))
    ppool = ctx.enter_context(tc.tile_pool(name="psum", bufs=2, space="PSUM"))

    # ---- biases -> [128, 1] per-partition tiles -------------------------
    bias_sb = {}
    for bname, flen in BIAS_SHAPES.items():
        nb = flen // PT
        bias_sb[bname] = []
        for p in range(nb):
            t = cpool.tile([PT, 1], F32, tag=f"{bname}_{p}", bufs=1, name=f"{bname}_{p}")
            nc.sync.dma_start(out=t, in_=prm[bname][p * PT:(p + 1) * PT][:, None])
            bias_sb[bname].append(t)

    # ---- weights -> float32r k-tiles [128, out_f] -----------------------
    w_r = {}
    for wname, (fin, fout) in WEIGHT_SHAPES.items():
        nk = fin // PT
        w_r[wname] = []
        for k in range(nk):
            stage = wpool.tile([PT, fout], F32, tag="wstage", bufs=3, name=f"stg_{wname}_{k}")
            nc.sync.dma_start(out=stage, in_=prm[wname][k * PT:(k + 1) * PT, :])
            t = cpool.tile([PT, fout], F32R, tag=f"{wname}_{k}", bufs=1, name=f"{wname}r_{k}")
            nc.vector.tensor_copy(t, stage)
            w_r[wname].append(t)

    # ---- objs -> objsT_r [feat, (n, b)] in float32r ---------------------
    objsT_r = []
    for p in range(D // PT):
        stage = []
        # strided transpose DMA, split over n for queue parallelism
        for nchunk in range(4):
            s = wpool.tile([PT, 4 * BL], F32, tag="ostage", bufs=8, name=f"ostg_{p}_{nchunk}")
            nc.sync.dma_start(
                out=s,
                in_=prm["objs"][4 * nchunk:4 * (nchunk + 1), :, p * PT:(p + 1) * PT]
                .rearrange("n b f -> f (n b)"))
            stage.append(s)
        t = cpool.tile([PT, ROWS], F32R, tag=f"objsT_{p}", bufs=1, name=f"objsTr_{p}")
        for nchunk in range(4):
            nc.vector.tensor_copy(t[:, 4 * BL * nchunk:4 * BL * (nchunk + 1)], stage[nchunk])
        objsT_r.append(t)

    zeros = cpool.tile([PT, 1], F32, tag="zeros", bufs=1, name="zeros")
    nc.gpsimd.memset(zeros, 0.0)

    # ---- generic dense layer: out[m][:, ncol] = drain(sum_k W[k,m].T @ rhs[k]) ----
    def layer(wname, rhs, drain, psum_tag, psum_bufs):
        wts = w_r[wname]
        nk = len(wts)
        fout = wts[0].shape[-1]
        nm = fout // PT
        ncols = rhs[0].shape[-1]
        for m in range(nm):
            for c in range(ncols // NT):
                ps = ppool.tile([PT, NT], F32, tag=psum_tag, bufs=psum_bufs,
                                name=f"ps_{wname}_{m}_{c}")
                for k in range(nk):
                    nc.tensor.matmul(
                        ps, wts[k][:, m * PT:(m + 1) * PT],
                        rhs[k][:, c * NT:(c + 1) * NT],
                        start=(k == 0), stop=(k == nk - 1))
                drain(m, c, ps)

    # ---- phase 1: U', V, trans MLP (rows = (n, b), 1024) ----------------
    def persist(tag, n_tiles, dt=F32, cols=ROWS):
        return [cpool.tile([PT, cols], dt, tag=f"{tag}_{m}", bufs=1, name=f"{tag}_{m}")
                for m in range(n_tiles)]

    U = persist("U", 4)      # objs @ iW1[:256] + ib1   (fp32, DVE-read only)
    V = persist("V", 4)      # objs @ iW1[256:]          (fp32, DVE-read only)
    t1r = persist("t1r", 4, F32R)
    t2r = persist("t2r", 4, F32R)
    t3 = persist("t3", 2)    # becomes S = t3 + objs after in-place add

    # U uses iW1 k-tiles 0..1 (obj_i features), V uses k-tiles 2..3 (obj_j)
    w_r["iW1a"] = w_r["iW1"][:2]
    w_r["iW1b"] = w_r["iW1"][2:]

    layer("iW1a", objsT_r,
          lambda m, c, ps: nc.scalar.activation(
              U[m][:, c * NT:(c + 1) * NT], ps, IDENT, bias=bias_sb["ib1"][m]),
          "ps2", 6)
    layer("iW1b", objsT_r,
          lambda m, c, ps: nc.vector.tensor_copy(V[m][:, c * NT:(c + 1) * NT], ps),
          "ps2", 6)
    layer("tW1", objsT_r,
          lambda m, c, ps: nc.scalar.activation(
              t1r[m][:, c * NT:(c + 1) * NT], ps, RELU, bias=bias_sb["tb1"][m]),
          "ps2", 6)
    layer("tW2", t1r,
          lambda m, c, ps: nc.scalar.activation(
              t2r[m][:, c * NT:(c + 1) * NT], ps, RELU, bias=bias_sb["tb2"][m]),
          "ps2", 6)
    layer("tW3", t2r,
          lambda m, c, ps: nc.vector.scalar_tensor_tensor(
              t3[m][:, c * NT:(c + 1) * NT], ps, bias_sb["tb3"][m],
              zeros.broadcast_to([PT, NT]), ADD, MAX),
          "ps2", 6)
    # S = t3 + objs  (in place)
    for p in range(2):
        nc.vector.tensor_add(t3[p], t3[p], objsT_r[p].bitcast(F32))

    # ---- phase 2: pair loop --------------------------------------------
    # rows of a tile: (i fixed, j in [jh*8, jh*8+8), b in [0,64))
    for i in range(N):
        it3 = [wpool.tile([PT, ROWS], F32, tag=f"inter3_{p}", bufs=2,
                          name=f"it3_{i}_{p}") for p in range(2)]
        for jh in range(2):
            cs = slice(jh * NT, (jh + 1) * NT)
            # h1 = U_i (broadcast over j) + V ; h1r = relu(h1) in f32r
            h1 = [wpool.tile([PT, NT], F32, tag=f"h1_{p}", bufs=2,
                             name=f"h1_{i}_{jh}_{p}") for p in range(4)]
            h1r = [wpool.tile([PT, NT], F32R, tag=f"h1r_{p}", bufs=2,
                              name=f"h1r_{i}_{jh}_{p}") for p in range(4)]
            for p in range(4):
                nc.vector.tensor_add(
                    h1[p].rearrange("p (j b) -> p j b", j=8),
                    V[p][:, cs].rearrange("p (j b) -> p j b", j=8),
                    U[p][:, None, i * BL:(i + 1) * BL].broadcast_to([PT, 8, BL]))
                nc.scalar.activation(h1r[p], h1[p], RELU)
            # L2
            xT2 = [wpool.tile([PT, NT], F32R, tag=f"xT2_{m}", bufs=2,
                              name=f"xT2_{i}_{jh}_{m}") for m in range(4)]
            for m in range(4):
                ps = ppool.tile([PT, NT], F32, tag="ps2", bufs=6,
                                name=f"psL2_{i}_{jh}_{m}")
                for k in range(4):
                    nc.tensor.matmul(ps, w_r["iW2"][k][:, m * PT:(m + 1) * PT],
                                     h1r[k], start=(k == 0), stop=(k == 3))
                if m % 2 == 0:
                    nc.scalar.activation(xT2[m], ps, RELU, bias=bias_sb["ib2"][m])
                else:
                    nc.vector.scalar_tensor_tensor(
                        xT2[m], ps, bias_sb["ib2"][m],
                        zeros.broadcast_to([PT, NT]), ADD, MAX)
            # L3
            for m in range(2):
                ps = ppool.tile([PT, NT], F32, tag="ps3", bufs=2,
                                name=f"psL3_{i}_{jh}_{m}")
                for k in range(4):
                    nc.tensor.matmul(ps, w_r["iW3"][k][:, m * PT:(m + 1) * PT],
                                     xT2[k], start=(k == 0), stop=(k == 3))
                if m == 0:
                    nc.scalar.activation(it3[m][:, cs], ps, RELU, bias=bias_sb["ib3"][m])
                else:
                    nc.vector.scalar_tensor_tensor(
                        it3[m][:, cs], ps, bias_sb["ib3"][m],
                        zeros.broadcast_to([PT, NT]), ADD, MAX)
        # zero the diagonal (j == i) block, then tree-reduce over j
        for p in range(2):
            nc.gpsimd.memset(it3[p][:, i * BL:(i + 1) * BL], 0.0)
        for p in range(2):
            acc = wpool.tile([PT, NT], F32, tag=f"acc_{p}", bufs=2,
                             name=f"acc_{i}_{p}")
            nc.vector.tensor_add(acc, it3[p][:, 0:512], it3[p][:, 512:1024])
            nc.vector.tensor_add(acc[:, 0:256], acc[:, 0:256], acc[:, 256:512])
            nc.vector.tensor_add(acc[:, 0:128], acc[:, 0:128], acc[:, 128:256])
            nc.vector.tensor_add(acc[:, 0:64], acc[:, 0:64], acc[:, 64:128])
            osb = wpool.tile([PT, BL], F32, tag=f"osb_{p}", bufs=2,
                             name=f"osb_{i}_{p}")
            nc.vector.tensor_add(osb, acc[:, 0:64], t3[p][:, i * BL:(i + 1) * BL])
            nc.sync.dma_start(
                out=prm["out"][i, :, p * PT:(p + 1) * PT].rearrange("b f -> f b"),
                in_=osb)


def build_nc():
    nc = bacc.Bacc("TRN2", target_bir_lowering=False, debug=False)
    prm = {}
    prm["objs"] = nc.declare_dram_parameter("objs", [N, BL, D], F32, isOutput=False)
    for wname, (fin, fout) in WEIGHT_SHAPES.items():
        prm[wname] = nc.declare_dram_parameter(wname, [fin, fout], F32, isOutput=False)
    for bname, flen in BIAS_SHAPES.items():
        prm[bname] = nc.declare_dram_parameter(bname, [flen], F32, isOutput=False)
    prm["out"] = nc.declare_dram_parameter("out", [N, BL, D], F32, isOutput=True)
    from contextlib import ExitStack
    with tile.TileContext(nc) as tc:
        with ExitStack() as ctx:
            _build_body(nc, tc, prm, ctx)
    nc.compile()
    return nc


_CACHE = {}


def _get_nc():
    if "nc" not in _CACHE:
        _CACHE["nc"] = build_nc()
    return _CACHE["nc"]


def make_in_maps(inputs):
    shared = {}
    for name in list(WEIGHT_SHAPES) + list(BIAS_SHAPES):
        shared[name] = np.ascontiguousarray(np.asarray(inputs[name], dtype=np.float32))
    objs = np.asarray(inputs["objs"], dtype=np.float32)
    in_maps = []
    for c in range(NCORES):
        m = dict(shared)
        m["objs"] = np.ascontiguousarray(objs[:, c * BL:(c + 1) * BL, :])
        in_maps.append(m)
    return in_maps


def kernel(**inputs):
    nc = _get_nc()
    in_maps = make_in_maps(inputs)
    res = run_bass_kernel_spmd(nc, in_maps, list(range(NCORES)))
    outs = [res.results[c]["out"] for c in range(NCORES)]
    return np.concatenate(outs, axis=1)


# revision 8
# speedup vs baseline: 1.0666x; 1.0666x over previous
"""Trainium2 Bass kernel for the interaction-network GNN (nn_Physics_7370163880185).

Reference computation (per batch element b, objects i=0..15, D=256):
  trans  = MLP_t(objs)                              # 256->512->512->256, relu x3
  pair(i,j) = concat(objs_i, objs_j)                # [512]
  inter  = MLP_i(pair)                              # 512->512->512->256, relu x3
  out    = trans + sum_{j != i} inter(i,j) + objs

Sharding: data-parallel over batch B=512 across 8 cores (64 per core).

Kernel strategy (per core):
  * Everything is computed in a feature-on-partition layout: activations are
    [feat, rows] with rows = (n, b) flattened; matmul(out, lhsT=W[k,m],
    rhs=xT[k, rows]) chains layers without transposes.
  * Interaction layer 1 is split: concat(a,b) @ iW1 = a @ iW1[:256] + b @ iW1[256:],
    so U = objs @ iW1[:256] and V = objs @ iW1[256:] are computed once on
    N*B rows instead of N^2*B rows; h1(i,j) = relu(U_i + V_j + ib1) is a
    broadcast add on the vector engine.
  * Matmuls run in float32r (full-rate fp32 on TRN2 PE for free dim >= 256).
  * The j-sum uses a log2 tree reduction with the diagonal block zeroed.
"""

import numpy as np

import concourse.bass as bass
import concourse.mybir as mybir
import concourse.tile as tile
from concourse import bacc
from concourse.bass_utils import run_bass_kernel_spmd

N = 16
B = 512
D = 256
NCORES = 8
BL = B // NCORES          # 64 batch rows per core
ROWS = N * BL             # 1024 (n, b) rows per core
PT = 128                  # partition tile
NT = 512                  # moving-dim (free) tile for matmuls

F32 = mybir.dt.float32
F32R = mybir.dt.float32r
RELU = mybir.ActivationFunctionType.Relu
IDENT = mybir.ActivationFunctionType.Identity
ADD = mybir.AluOpType.add
MAX = mybir.AluOpType.max

WEIGHT_SHAPES = {
    "tW1": (256, 512), "tW2": (512, 512), "tW3": (512, 256),
    "iW1": (512, 512), "iW2": (512, 512), "iW3": (512, 256),
}
BIAS_SHAPES = {"tb1": 512, "tb2": 512, "tb3": 256,
               "ib1": 512, "ib2": 512, "ib3": 256}


ALL_PARTS = frozenset({"in_dma", "w_dma", "phase1", "h1", "mm2", "drain2", "reduce", "out_dma"})


def _build_body(nc, tc, prm, ctx, parts=ALL_PARTS):
    cpool = ctx.enter_context(tc.tile_pool(name="const", bufs=1))
    wpool = ctx.enter_context(tc.tile_pool(name="work", bufs=2))
    ppool = ctx.enter_context(tc.tile_pool(name="psum", bufs=2, space="PSUM"))

    # ---- biases -> [128, 1] per-partition tiles -------------------------
    bias_sb = {}
    for bname, flen in BIAS_SHAPES.items():
        nb = flen // PT
        bias_sb[bname] = []
        for p in range(nb):
            t = cpool.tile([PT, 1], F32, tag=f"{bname}_{p}", bufs=1, name=f"{bname}_{p}")
            nc.sync.dma_start(out=t, in_=prm[bname][p * PT:(p + 1) * PT][:, None])
            bias_sb[bname].append(t)

    # ---- weights -> float32r k-tiles [128, out_f] -----------------------
    w_r = {}
    for wname, (fin, fout) in WEIGHT_SHAPES.items():
        nk = fin // PT
        w_r[wname] = []
        use_w = ("mm2" in parts) or ("phase1" in parts)
        for k in range(nk):
            t = cpool.tile([PT, fout], F32R, tag=f"{wname}_{k}", bufs=1, name=f"{wname}r_{k}")
            if "w_dma" in parts and use_w:
                stage = wpool.tile([PT, fout], F32, tag="wstage", bufs=3, name=f"stg_{wname}_{k}")
                nc.sync.dma_start(out=stage, in_=prm[wname][k * PT:(k + 1) * PT, :])
                nc.vector.tensor_copy(t, stage)
            w_r[wname].append(t)

    # ---- objs -> objsT_r [feat, (n, b)] in float32r ---------------------
    objsT_r = []
    for p in range(D // PT):
        t = cpool.tile([PT, ROWS], F32R, tag=f"objsT_{p}", bufs=1, name=f"objsTr_{p}")
        if "in_dma" in parts and "phase1" in parts:
            s = wpool.tile([PT, ROWS], F32, tag="ostage", bufs=2, name=f"ostg_{p}")
            nc.sync.dma_start(out=s, in_=prm["objs"][p * PT:(p + 1) * PT, :])
            nc.vector.tensor_copy(t, s)
        objsT_r.append(t)

    zeros = cpool.tile([PT, 1], F32, tag="zeros", bufs=1, name="zeros")
    nc.gpsimd.memset(zeros, 0.0)

    # ---- generic dense layer: out[m][:, ncol] = drain(sum_k W[k,m].T @ rhs[k]) ----
    def layer(wname, rhs, drain, psum_tag, psum_bufs):
        wts = w_r[wname]
        nk = len(wts)
        fout = wts[0].shape[-1]
        nm = fout // PT
        ncols = rhs[0].shape[-1]
        for m in range(nm):
            for c in range(ncols // NT):
                ps = ppool.tile([PT, NT], F32, tag=psum_tag, bufs=psum_bufs,
                                name=f"ps_{wname}_{m}_{c}")
                for k in range(nk):
                    nc.tensor.matmul(
                        ps, wts[k][:, m * PT:(m + 1) * PT],
                        rhs[k][:, c * NT:(c + 1) * NT],
                        start=(k == 0), stop=(k == nk - 1))
                drain(m, c, ps)

    # ---- phase 1: U', V, trans MLP (rows = (n, b), 1024) ----------------
    def persist(tag, n_tiles, dt=F32, cols=ROWS):
        return [cpool.tile([PT, cols], dt, tag=f"{tag}_{m}", bufs=1, name=f"{tag}_{m}")
                for m in range(n_tiles)]

    U = persist("U", 4)      # objs @ iW1[:256] + ib1   (fp32, DVE-read only)
    V = persist("V", 4)      # objs @ iW1[256:]          (fp32, DVE-read only)
    t1r = persist("t1r", 4, F32R)
    t2r = persist("t2r", 4, F32R)
    t3 = persist("t3", 2)    # becomes S = t3 + objs after in-place add

    # U uses iW1 k-tiles 0..1 (obj_i features), V uses k-tiles 2..3 (obj_j)
    w_r["iW1a"] = w_r["iW1"][:2]
    w_r["iW1b"] = w_r["iW1"][2:]

    if "phase1" not in parts:
        pass
    else:
      layer("iW1a", objsT_r,
          lambda m, c, ps: nc.scalar.activation(
              U[m][:, c * NT:(c + 1) * NT], ps, IDENT, bias=bias_sb["ib1"][m]),
          "ps2", 6)
      layer("iW1b", objsT_r,
          lambda m, c, ps: nc.vector.tensor_copy(V[m][:, c * NT:(c + 1) * NT], ps),
          "ps2", 6)
      layer("tW1", objsT_r,
          lambda m, c, ps: nc.scalar.activation(
              t1r[m][:, c * NT:(c + 1) * NT], ps, RELU, bias=bias_sb["tb1"][m]),
          "ps2", 6)
      layer("tW2", t1r,
          lambda m, c, ps: nc.scalar.activation(
              t2r[m][:, c * NT:(c + 1) * NT], ps, RELU, bias=bias_sb["tb2"][m]),
          "ps2", 6)
      layer("tW3", t2r,
          lambda m, c, ps: nc.vector.scalar_tensor_tensor(
              t3[m][:, c * NT:(c + 1) * NT], ps, bias_sb["tb3"][m],
              zeros.broadcast_to([PT, NT]), ADD, MAX),
          "ps2", 6)
      # S = t3 + objs  (in place)
      for p in range(2):
        nc.vector.tensor_add(t3[p], t3[p], objsT_r[p].bitcast(F32))

    # ---- phase 2: pair loop --------------------------------------------
    # rows of a tile: (i fixed, j in [jh*8, jh*8+8), b in [0,64))
    for i in range(N):
        it3 = [wpool.tile([PT, ROWS], F32, tag=f"inter3_{p}", bufs=2,
                          name=f"it3_{i}_{p}") for p in range(2)]
        for jh in range(2):
            cs = slice(jh * NT, (jh + 1) * NT)
            # h1 = U_i (broadcast over j) + V ; h1r = relu(h1) in f32r
            h1 = [wpool.tile([PT, NT], F32, tag=f"h1_{p}", bufs=2,
                             name=f"h1_{i}_{jh}_{p}") for p in range(4)]
            h1r = [wpool.tile([PT, NT], F32R, tag=f"h1r_{p}", bufs=2,
                              name=f"h1r_{i}_{jh}_{p}") for p in range(4)]
            if "h1" in parts:
                for p in range(4):
                    nc.vector.tensor_add(
                        h1[p].rearrange("p (j b) -> p j b", j=8),
                        V[p][:, cs].rearrange("p (j b) -> p j b", j=8),
                        U[p][:, None, i * BL:(i + 1) * BL].broadcast_to([PT, 8, BL]))
                    nc.scalar.activation(h1r[p], h1[p], RELU)
            # L2
            xT2 = [wpool.tile([PT, NT], F32R, tag=f"xT2_{m}", bufs=2,
                              name=f"xT2_{i}_{jh}_{m}") for m in range(4)]
            for m in range(4):
                ps = ppool.tile([PT, NT], F32, tag="ps2", bufs=6,
                                name=f"psL2_{i}_{jh}_{m}")
                if "mm2" in parts:
                    for k in range(4):
                        nc.tensor.matmul(ps, w_r["iW2"][k][:, m * PT:(m + 1) * PT],
                                         h1r[k], start=(k == 0), stop=(k == 3))
                if "drain2" in parts:
                    if m % 2 == 0:
                        nc.scalar.activation(xT2[m], ps, RELU, bias=bias_sb["ib2"][m])
                    else:
                        nc.vector.scalar_tensor_tensor(
                            xT2[m], ps, bias_sb["ib2"][m],
                            zeros.broadcast_to([PT, NT]), ADD, MAX)
            # L3
            for m in range(2):
                ps = ppool.tile([PT, NT], F32, tag="ps3", bufs=2,
                                name=f"psL3_{i}_{jh}_{m}")
                if "mm2" in parts:
                    for k in range(4):
                        nc.tensor.matmul(ps, w_r["iW3"][k][:, m * PT:(m + 1) * PT],
                                         xT2[k], start=(k == 0), stop=(k == 3))
                if "drain2" in parts:
                    if m == 0:
                        nc.scalar.activation(it3[m][:, cs], ps, RELU, bias=bias_sb["ib3"][m])
                    else:
                        nc.vector.scalar_tensor_tensor(
                            it3[m][:, cs], ps, bias_sb["ib3"][m],
                            zeros.broadcast_to([PT, NT]), ADD, MAX)
        # zero the diagonal (j == i) block, then tree-reduce over j
        if "reduce" not in parts:
            continue
        for p in range(2):
            nc.gpsimd.memset(it3[p][:, i * BL:(i + 1) * BL], 0.0)
        for p in range(2):
            acc = wpool.tile([PT, NT], F32, tag=f"acc_{p}", bufs=2,
                             name=f"acc_{i}_{p}")
            nc.vector.tensor_add(acc, it3[p][:, 0:512], it3[p][:, 512:1024])
            nc.vector.tensor_add(acc[:, 0:256], acc[:, 0:256], acc[:, 256:512])
            nc.vector.tensor_add(acc[:, 0:128], acc[:, 0:128], acc[:, 128:256])
            nc.vector.tensor_add(acc[:, 0:64], acc[:, 0:64], acc[:, 64:128])
            osb = wpool.tile([PT, BL], F32, tag=f"osb_{p}", bufs=2,
                             name=f"osb_{i}_{p}")
            nc.vector.tensor_add(osb, acc[:, 0:64], t3[p][:, i * BL:(i + 1) * BL])
            if "out_dma" in parts:
                nc.sync.dma_start(out=prm["out"][p * PT:(p + 1) * PT, i, :], in_=osb)


def build_nc(loop_iters=None, parts=ALL_PARTS):
    """loop_iters: if set, wrap the whole body in a hardware For_i loop that
    repeats it that many times (used only for timing measurements)."""
    nc = bacc.Bacc("TRN2", target_bir_lowering=False, debug=False)
    prm = {}
    prm["objs"] = nc.declare_dram_parameter("objs", [D, ROWS], F32, isOutput=False)
    for wname, (fin, fout) in WEIGHT_SHAPES.items():
        prm[wname] = nc.declare_dram_parameter(wname, [fin, fout], F32, isOutput=False)
    for bname, flen in BIAS_SHAPES.items():
        prm[bname] = nc.declare_dram_parameter(bname, [flen], F32, isOutput=False)
    prm["out"] = nc.declare_dram_parameter("out", [D, N, BL], F32, isOutput=True)
    from contextlib import ExitStack
    with tile.TileContext(nc) as tc:
        if loop_iters is None:
            with ExitStack() as ctx:
                _build_body(nc, tc, prm, ctx, parts)
        else:
            with tc.For_i(0, loop_iters, 1):
                with ExitStack() as ctx:
                    _build_body(nc, tc, prm, ctx, parts)
    nc.compile()
    return nc


_CACHE = {}


def _get_nc():
    if "nc" not in _CACHE:
        _CACHE["nc"] = build_nc()
    return _CACHE["nc"]


def make_in_maps(inputs):
    shared = {}
    for name in list(WEIGHT_SHAPES) + list(BIAS_SHAPES):
        shared[name] = np.ascontiguousarray(np.asarray(inputs[name], dtype=np.float32))
    objs = np.asarray(inputs["objs"], dtype=np.float32)
    in_maps = []
    for c in range(NCORES):
        m = dict(shared)
        sl = objs[:, c * BL:(c + 1) * BL, :]            # [N, BL, D]
        m["objs"] = np.ascontiguousarray(sl.transpose(2, 0, 1).reshape(D, ROWS))
        in_maps.append(m)
    return in_maps


def kernel(**inputs):
    nc = _get_nc()
    in_maps = make_in_maps(inputs)
    res = run_bass_kernel_spmd(nc, in_maps, list(range(NCORES)))
    outs = [res.results[c]["out"].transpose(1, 2, 0) for c in range(NCORES)]  # -> [N, BL, D]
    return np.concatenate(outs, axis=1)


# revision 16
# speedup vs baseline: 3832.1934x; 3593.0369x over previous
"""Trainium2 Bass kernel for the interaction-network GNN (nn_Physics_7370163880185).

Reference computation (per batch element b, objects i=0..15, D=256):
  trans  = MLP_t(objs)                              # 256->512->512->256, relu x3
  pair(i,j) = concat(objs_i, objs_j)                # [512]
  inter  = MLP_i(pair)                              # 512->512->512->256, relu x3
  out    = trans + sum_{j != i} inter(i,j) + objs

Sharding: data-parallel over batch B=512 across 8 cores (64 per core).

Kernel strategy (per core):
  * Everything is computed in a feature-on-partition layout: activations are
    [feat, rows] with rows = (n, b) flattened; matmul(out, lhsT=W[k,m],
    rhs=xT[k, rows]) chains layers without transposes.
  * Interaction layer 1 is split: concat(a,b) @ iW1 = a @ iW1[:256] + b @ iW1[256:],
    so U = objs @ iW1[:256] and V = objs @ iW1[256:] are computed once on
    N*B rows instead of N^2*B rows; h1(i,j) = relu(U_i + V_j + ib1) is a
    broadcast add on the vector engine.
  * Matmuls run in float32r (full-rate fp32 on TRN2 PE for free dim >= 256).
  * The j-sum uses a log2 tree reduction with the diagonal block zeroed.
"""

import numpy as np

import concourse.bass as bass
import concourse.mybir as mybir
import concourse.tile as tile
from concourse import bacc
from concourse.bass_utils import run_bass_kernel_spmd

N = 16
B = 512
D = 256
NCORES = 8
BL = B // NCORES          # 64 batch rows per core
ROWS = N * BL             # 1024 (n, b) rows per core
PT = 128                  # partition tile
NT = 512                  # moving-dim (free) tile for matmuls

F32 = mybir.dt.float32
F32R = mybir.dt.float32r
BF16 = mybir.dt.bfloat16
RELU = mybir.ActivationFunctionType.Relu
IDENT = mybir.ActivationFunctionType.Identity
ADD = mybir.AluOpType.add
MAX = mybir.AluOpType.max

WEIGHT_SHAPES = {
    "tW1": (256, 512), "tW2": (512, 512), "tW3": (512, 256),
    "iW1": (512, 512), "iW2": (512, 512), "iW3": (512, 256),
}
BIAS_SHAPES = {"tb1": 512, "tb2": 512, "tb3": 256,
               "ib1": 512, "ib2": 512, "ib3": 256}


ALL_PARTS = frozenset({"in_dma", "w_dma", "phase1", "h1", "mm2", "drain2", "reduce", "out_dma"})


def _build_body(nc, tc, prm, ctx, parts=ALL_PARTS, dup=None, mm_dt=F32R, h1_mode="bcast", ilv=True, paired=False):
    dup = dup or {}
    def rep(key):
        return range(dup.get(key, 1))
    cpool = ctx.enter_context(tc.tile_pool(name="const", bufs=1))
    wpool = ctx.enter_context(tc.tile_pool(name="work", bufs=2))
    ppool = ctx.enter_context(tc.tile_pool(name="psum", bufs=2, space="PSUM"))

    # ---- biases -> [128, 1] per-partition tiles -------------------------
    bias_sb = {}
    for bname, flen in BIAS_SHAPES.items():
        nb = flen // PT
        bias_sb[bname] = []
        for p in range(nb):
            t = cpool.tile([PT, 1], F32, tag=f"{bname}_{p}", bufs=1, name=f"{bname}_{p}")
            nc.sync.dma_start(out=t, in_=prm[bname][p * PT:(p + 1) * PT][:, None])
            bias_sb[bname].append(t)

    # ---- weights -> float32r k-tiles [128, out_f] -----------------------
    w_r = {}
    for wname, (fin, fout) in WEIGHT_SHAPES.items():
        nk = fin // PT
        w_r[wname] = []
        use_w = ("mm2" in parts) or ("phase1" in parts)
        for k in range(nk):
            t = cpool.tile([PT, fout], mm_dt, tag=f"{wname}_{k}", bufs=1, name=f"{wname}r_{k}")
            if "w_dma" in parts and use_w:
                stage = wpool.tile([PT, fout], F32, tag="wstage", bufs=3, name=f"stg_{wname}_{k}")
                nc.sync.dma_start(out=stage, in_=prm[wname][k * PT:(k + 1) * PT, :])
                nc.vector.tensor_copy(t, stage)
            w_r[wname].append(t)

    # ---- objs -> objsT_r [feat, (n, b)] in float32r ---------------------
    objsT_r = []
    for p in range(D // PT):
        t = cpool.tile([PT, ROWS], mm_dt, tag=f"objsT_{p}", bufs=1, name=f"objsTr_{p}")
        if "in_dma" in parts and "phase1" in parts:
            s = wpool.tile([PT, ROWS], F32, tag="ostage", bufs=2, name=f"ostg_{p}")
            nc.sync.dma_start(out=s, in_=prm["objs"][p * PT:(p + 1) * PT, :])
            nc.vector.tensor_copy(t, s)
        objsT_r.append(t)

    zeros = cpool.tile([PT, 1], F32, tag="zeros", bufs=1, name="zeros")
    nc.gpsimd.memset(zeros, 0.0)

    # ---- generic dense layer: out[m][:, ncol] = drain(sum_k W[k,m].T @ rhs[k]) ----
    # Matmuls for a pair of (m, c) outputs are interleaved k-outer so that
    # consecutive PE instructions target different PSUM banks.
    def layer(wname, rhs, drain, psum_tag, psum_bufs):
        wts = w_r[wname]
        nk = len(wts)
        fout = wts[0].shape[-1]
        nm = fout // PT
        ncols = rhs[0].shape[-1]
        mc = [(m, c) for m in range(nm) for c in range(ncols // NT)]
        if not ilv:
            for m, c in mc:
                ps = ppool.tile([PT, NT], F32, tag=psum_tag, bufs=psum_bufs,
                                name=f"ps_{wname}_{m}_{c}")
                for k in range(nk):
                    nc.tensor.matmul(
                        ps, wts[k][:, m * PT:(m + 1) * PT],
                        rhs[k][:, c * NT:(c + 1) * NT],
                        start=(k == 0), stop=(k == nk - 1))
                drain(m, c, ps)
            return
        for g in range(0, len(mc), 2):
            grp = mc[g:g + 2]
            pss = [ppool.tile([PT, NT], F32, tag=psum_tag, bufs=psum_bufs,
                              name=f"ps_{wname}_{m}_{c}") for (m, c) in grp]
            for k in range(nk):
                for (m, c), ps in zip(grp, pss):
                    nc.tensor.matmul(
                        ps, wts[k][:, m * PT:(m + 1) * PT],
                        rhs[k][:, c * NT:(c + 1) * NT],
                        start=(k == 0), stop=(k == nk - 1))
            for (m, c), ps in zip(grp, pss):
                drain(m, c, ps)

    # ---- phase 1: U', V, trans MLP (rows = (n, b), 1024) ----------------
    def persist(tag, n_tiles, dt=F32, cols=ROWS):
        return [cpool.tile([PT, cols], dt, tag=f"{tag}_{m}", bufs=1, name=f"{tag}_{m}")
                for m in range(n_tiles)]

    ps2b = 8 if paired else 6
    U = persist("U", 4)      # objs @ iW1[:256] + ib1   (fp32, DVE-read only)
    V = persist("V", 4)      # objs @ iW1[256:]          (fp32, DVE-read only)
    t1r = persist("t1r", 4, mm_dt)
    t2r = persist("t2r", 4, mm_dt)
    t3 = persist("t3", 2)    # becomes S = t3 + objs after in-place add

    # U uses iW1 k-tiles 0..1 (obj_i features), V uses k-tiles 2..3 (obj_j)
    w_r["iW1a"] = w_r["iW1"][:2]
    w_r["iW1b"] = w_r["iW1"][2:]

    if "phase1" not in parts:
        pass
    else:
      layer("iW1a", objsT_r,
          lambda m, c, ps: nc.scalar.activation(
              U[m][:, c * NT:(c + 1) * NT], ps, IDENT, bias=bias_sb["ib1"][m]),
          "ps2", ps2b)
      layer("iW1b", objsT_r,
          lambda m, c, ps: nc.vector.tensor_copy(V[m][:, c * NT:(c + 1) * NT], ps),
          "ps2", ps2b)
      layer("tW1", objsT_r,
          lambda m, c, ps: nc.scalar.activation(
              t1r[m][:, c * NT:(c + 1) * NT], ps, RELU, bias=bias_sb["tb1"][m]),
          "ps2", ps2b)
      layer("tW2", t1r,
          lambda m, c, ps: nc.scalar.activation(
              t2r[m][:, c * NT:(c + 1) * NT], ps, RELU, bias=bias_sb["tb2"][m]),
          "ps2", ps2b)
      layer("tW3", t2r,
          lambda m, c, ps: nc.vector.tensor_scalar(
              t3[m][:, c * NT:(c + 1) * NT], ps, bias_sb["tb3"][m], 0.0, ADD, MAX),
          "ps2", ps2b)
      # S = t3 + objs  (in place)
      for p in range(2):
        ob = objsT_r[p].bitcast(F32) if objsT_r[p].dtype == F32R else objsT_r[p]
        nc.vector.tensor_add(t3[p], t3[p], ob)

    # ---- phase 2 (jh-paired variant) -----------------------------------
    def phase2_paired():
        for i in range(N):
            it3 = [wpool.tile([PT, ROWS], F32, tag=f"inter3_{p}", bufs=2,
                              name=f"it3_{i}_{p}") for p in range(2)]
            h1r2 = []
            for jh in range(2):
                cs = slice(jh * NT, (jh + 1) * NT)
                h1 = [wpool.tile([PT, NT], F32, tag=f"h1_{jh}_{p}", bufs=1,
                                 name=f"h1_{i}_{jh}_{p}") for p in range(4)]
                h1r = [wpool.tile([PT, NT], mm_dt, tag=f"h1r_{jh}_{p}", bufs=1,
                                  name=f"h1r_{i}_{jh}_{p}") for p in range(4)]
                for p in range(4):
                    nc.vector.tensor_add(
                        h1[p].rearrange("p (j b) -> p j b", j=8),
                        V[p][:, cs].rearrange("p (j b) -> p j b", j=8),
                        U[p][:, None, i * BL:(i + 1) * BL].broadcast_to([PT, 8, BL]))
                    nc.scalar.activation(h1r[p], h1[p], RELU)
                h1r2.append(h1r)
            # L2: m-pairs x jh, weights shared by consecutive MMs (jh inner)
            xT22 = [[wpool.tile([PT, NT], mm_dt, tag=f"xT2_{jh}_{m}", bufs=1,
                                name=f"xT2_{i}_{jh}_{m}") for m in range(4)]
                    for jh in range(2)]
            for mh in range(2):
                ms = [2 * mh, 2 * mh + 1]
                pss = {(m, jh): ppool.tile([PT, NT], F32, tag="ps2", bufs=8,
                                           name=f"psL2_{i}_{jh}_{m}")
                       for m in ms for jh in range(2)}
                for k in range(4):
                    for m in ms:
                        for jh in range(2):
                            nc.tensor.matmul(pss[(m, jh)],
                                             w_r["iW2"][k][:, m * PT:(m + 1) * PT],
                                             h1r2[jh][k], start=(k == 0), stop=(k == 3))
                for m in ms:
                    for jh in range(2):
                        if m % 2 == 0:
                            nc.scalar.activation(xT22[jh][m], pss[(m, jh)], RELU,
                                                 bias=bias_sb["ib2"][m])
                        else:
                            nc.vector.tensor_scalar(
                                xT22[jh][m], pss[(m, jh)], bias_sb["ib2"][m], 0.0, ADD, MAX)
            # L3
            pss3 = {(m, jh): ppool.tile([PT, NT], F32, tag="ps2", bufs=8,
                                        name=f"psL3_{i}_{jh}_{m}")
                    for m in range(2) for jh in range(2)}
            for k in range(4):
                for m in range(2):
                    for jh in range(2):
                        nc.tensor.matmul(pss3[(m, jh)],
                                         w_r["iW3"][k][:, m * PT:(m + 1) * PT],
                                         xT22[jh][k], start=(k == 0), stop=(k == 3))
            for m in range(2):
                for jh in range(2):
                    cs = slice(jh * NT, (jh + 1) * NT)
                    if m == 0:
                        nc.scalar.activation(it3[m][:, cs], pss3[(m, jh)], RELU,
                                             bias=bias_sb["ib3"][m])
                    else:
                        nc.vector.tensor_scalar(
                            it3[m][:, cs], pss3[(m, jh)], bias_sb["ib3"][m], 0.0, ADD, MAX)
            # diagonal zero + reduce + output
            for p in range(2):
                nc.gpsimd.memset(it3[p][:, i * BL:(i + 1) * BL], 0.0)
            for p in range(2):
                acc = wpool.tile([PT, NT], F32, tag=f"acc_{p}", bufs=2,
                                 name=f"acc_{i}_{p}")
                nc.vector.tensor_add(acc, it3[p][:, 0:512], it3[p][:, 512:1024])
                nc.vector.tensor_add(acc[:, 0:256], acc[:, 0:256], acc[:, 256:512])
                nc.vector.tensor_add(acc[:, 0:128], acc[:, 0:128], acc[:, 128:256])
                nc.vector.tensor_add(acc[:, 0:64], acc[:, 0:64], acc[:, 64:128])
                osb = wpool.tile([PT, BL], F32, tag=f"osb_{p}", bufs=2,
                                 name=f"osb_{i}_{p}")
                nc.vector.tensor_add(osb, acc[:, 0:64], t3[p][:, i * BL:(i + 1) * BL])
                nc.sync.dma_start(out=prm["out"][p * PT:(p + 1) * PT, i, :], in_=osb)

    if paired:
        phase2_paired()
        return

    # ---- phase 2: pair loop --------------------------------------------
    # rows of a tile: (i fixed, j in [jh*8, jh*8+8), b in [0,64))
    for i in range(N):
        it3 = [wpool.tile([PT, ROWS], F32, tag=f"inter3_{p}", bufs=2,
                          name=f"it3_{i}_{p}") for p in range(2)] \
            if ("drain2" in parts or "reduce" in parts) else None
        for jh in range(2):
            cs = slice(jh * NT, (jh + 1) * NT)
            # h1 = U_i (broadcast over j) + V ; h1r = relu(h1) in f32r
            h1 = [wpool.tile([PT, NT], F32, tag=f"h1_{p}", bufs=2,
                             name=f"h1_{i}_{jh}_{p}") for p in range(4)] \
                if "h1" in parts else None
            h1r = [wpool.tile([PT, NT], mm_dt, tag=f"h1r_{p}", bufs=2,
                              name=f"h1r_{i}_{jh}_{p}") for p in range(4)] \
                if ("h1" in parts or "mm2" in parts) else None
            if "h1" in parts:
                for p in range(4):
                    for _ in rep("h1add"):
                        if h1_mode == "bcast":
                            nc.vector.tensor_add(
                                h1[p].rearrange("p (j b) -> p j b", j=8),
                                V[p][:, cs].rearrange("p (j b) -> p j b", j=8),
                                U[p][:, None, i * BL:(i + 1) * BL].broadcast_to([PT, 8, BL]))
                        else:  # "flat": timing-only, numerically wrong
                            nc.vector.tensor_add(h1[p], V[p][:, cs], U[p][:, cs])
                    for _ in rep("relu"):
                        nc.scalar.activation(h1r[p], h1[p], RELU)
            # L2
            xT2 = [wpool.tile([PT, NT], mm_dt, tag=f"xT2_{m}", bufs=2,
                              name=f"xT2_{i}_{jh}_{m}") for m in range(4)] \
                if ("drain2" in parts or "mm2" in parts) else None
            for mp in range(2):
                if not ({"mm2", "drain2"} & parts):
                    continue
                ms = [2 * mp, 2 * mp + 1]
                pss = [ppool.tile([PT, NT], F32, tag="ps2", bufs=6,
                                  name=f"psL2_{i}_{jh}_{m}") for m in ms]
                if "mm2" in parts:
                    for _ in rep("mm"):
                        for k in range(4):
                            for m, ps in zip(ms, pss):
                                nc.tensor.matmul(ps, w_r["iW2"][k][:, m * PT:(m + 1) * PT],
                                                 h1r[k], start=(k == 0), stop=(k == 3))
                if "drain2" in parts:
                    for _ in rep("drain"):
                        for m, ps in zip(ms, pss):
                            if m % 2 == 0:
                                nc.scalar.activation(xT2[m], ps, RELU, bias=bias_sb["ib2"][m])
                            else:
                                nc.vector.tensor_scalar(
                                    xT2[m], ps, bias_sb["ib2"][m], 0.0, ADD, MAX)
            # L3
            if {"mm2", "drain2"} & parts:
                pss3 = [ppool.tile([PT, NT], F32, tag="ps3", bufs=2,
                                   name=f"psL3_{i}_{jh}_{m}") for m in range(2)]
                if "mm2" in parts:
                    for _ in rep("mm"):
                        for k in range(4):
                            for m in range(2):
                                nc.tensor.matmul(pss3[m], w_r["iW3"][k][:, m * PT:(m + 1) * PT],
                                                 xT2[k], start=(k == 0), stop=(k == 3))
                if "drain2" in parts:
                    for _ in rep("drain"):
                        for m in range(2):
                            if m == 0:
                                nc.scalar.activation(it3[m][:, cs], pss3[m], RELU, bias=bias_sb["ib3"][m])
                            else:
                                nc.vector.tensor_scalar(
                                    it3[m][:, cs], pss3[m], bias_sb["ib3"][m], 0.0, ADD, MAX)
        # zero the diagonal (j == i) block, then tree-reduce over j
        if "reduce" not in parts:
            continue
        for p in range(2):
            nc.gpsimd.memset(it3[p][:, i * BL:(i + 1) * BL], 0.0)
        for p in range(2):
            acc = wpool.tile([PT, NT], F32, tag=f"acc_{p}", bufs=2,
                             name=f"acc_{i}_{p}")
            for _ in rep("reduce"):
                nc.vector.tensor_add(acc, it3[p][:, 0:512], it3[p][:, 512:1024])
            nc.vector.tensor_add(acc[:, 0:256], acc[:, 0:256], acc[:, 256:512])
            nc.vector.tensor_add(acc[:, 0:128], acc[:, 0:128], acc[:, 128:256])
            nc.vector.tensor_add(acc[:, 0:64], acc[:, 0:64], acc[:, 64:128])
            osb = wpool.tile([PT, BL], F32, tag=f"osb_{p}", bufs=2,
                             name=f"osb_{i}_{p}")
            nc.vector.tensor_add(osb, acc[:, 0:64], t3[p][:, i * BL:(i + 1) * BL])
            if "out_dma" in parts:
                nc.sync.dma_start(out=prm["out"][p * PT:(p + 1) * PT, i, :], in_=osb)


def build_nc(loop_iters=None, parts=ALL_PARTS, dup=None, mm_dt=F32R, h1_mode="bcast", ilv=True, paired=False):
    """loop_iters: if set, wrap the whole body in a hardware For_i loop that
    repeats it that many times (used only for timing measurements)."""
    nc = bacc.Bacc("TRN2", target_bir_lowering=False, debug=False)
    prm = {}
    prm["objs"] = nc.declare_dram_parameter("objs", [D, ROWS], F32, isOutput=False)
    for wname, (fin, fout) in WEIGHT_SHAPES.items():
        prm[wname] = nc.declare_dram_parameter(wname, [fin, fout], F32, isOutput=False)
    for bname, flen in BIAS_SHAPES.items():
        prm[bname] = nc.declare_dram_parameter(bname, [flen], F32, isOutput=False)
    prm["out"] = nc.declare_dram_parameter("out", [D, N, BL], F32, isOutput=True)
    from contextlib import ExitStack
    with tile.TileContext(nc) as tc:
        if loop_iters is None:
            with ExitStack() as ctx:
                _build_body(nc, tc, prm, ctx, parts, dup, mm_dt, h1_mode, ilv, paired)
        else:
            with tc.For_i(0, loop_iters, 1):
                with ExitStack() as ctx:
                    _build_body(nc, tc, prm, ctx, parts, dup, mm_dt, h1_mode, ilv, paired)
    nc.compile()
    return nc


_CACHE = {}


def _get_nc():
    if "nc" not in _CACHE:
        _CACHE["nc"] = build_nc()
    return _CACHE["nc"]


def make_in_maps(inputs):
    shared = {}
    for name in list(WEIGHT_SHAPES) + list(BIAS_SHAPES):
        shared[name] = np.ascontiguousarray(np.asarray(inputs[name], dtype=np.float32))
    objs = np.asarray(inputs["objs"], dtype=np.float32)
    in_maps = []
    for c in range(NCORES):
        m = dict(shared)
        sl = objs[:, c * BL:(c + 1) * BL, :]            # [N, BL, D]
        m["objs"] = np.ascontiguousarray(sl.transpose(2, 0, 1).reshape(D, ROWS))
        in_maps.append(m)
    return in_maps


def kernel(**inputs):
    nc = _get_nc()
    in_maps = make_in_maps(inputs)
    res = run_bass_kernel_spmd(nc, in_maps, list(range(NCORES)))
    outs = [res.results[c]["out"].transpose(1, 2, 0) for c in range(NCORES)]  # -> [N, BL, D]
    return np.concatenate(outs, axis=1)
